# revision 30
# baseline (speedup 1.0000x reference)
"""Additive-attention pooling (nn_Meta_Module) Trainium2 kernel.

Full inputs in, full output out. Internally: pure data-parallel over 8
NeuronCores (batch 512 -> 64/core). Per core, a Bass/Tile kernel computes
  a   = all_memory @ U.T            (PE, bf16, [k,(b,s)] layout)
  t   = tanh(a + last @ W.T)        (ScalarE, per-batch per-partition bias)
  sc  = V.T @ t                     (PE, V-stationary, out row = batch idx)
  P   = all_memory @ MetaW.T        (PE, [128,4] MW-stationary, 4-row blocks)
  e   = exp(sc), esum = accum       (ScalarE fused accum_out)
  numer = sum_s e*P                 (PE replicate + DVE fused mul-reduce)
host: numer/esum + Metab.
"""
import numpy as np
import ml_dtypes
from contextlib import ExitStack

import concourse.bass as bass
import concourse.tile as tile
import concourse.mybir as mybir
from concourse import bacc
from concourse.bass_utils import run_bass_kernel_spmd

BF16 = mybir.dt.bfloat16
F32 = mybir.dt.float32
F8 = mybir.dt.float8e4
AF = mybir.ActivationFunctionType
ALU = mybir.AluOpType
NBF = ml_dtypes.bfloat16
NF8 = ml_dtypes.float8_e4m3
DR = mybir.MatmulPerfMode.DoubleRow

B, S, H = 512, 200, 256
N_CORES = 8
B_LOC = B // N_CORES          # 64 batches per core
GROUP = 32                    # batches per PT tile (4 rows each)


def build_nc(b_loc=B_LOC, debug=False):
    ROUNDS = b_loc // 4
    nc = bacc.Bacc("TRN2", target_bir_lowering=False, debug=debug)

    def din(name, shape, dt=BF16):
        return nc.dram_tensor(name, shape, dt, kind="ExternalInput")

    allT = [din(f"allT{h}", [128, b_loc * S]) for h in range(2)]
    CBA_d = din("CBA", [128, 512])
    CBB_d = din("CBB", [128, 512 + 128])
    CBF8_d = din("CBF8", [128, 128, 2, 64], F8)
    LT_d = din("LT", [128, 2 * b_loc], F32)
    numer_d = nc.dram_tensor("numer", [128, 2], F32, kind="ExternalOutput")
    esum_d = nc.dram_tensor("esum", [b_loc, 1], F32, kind="ExternalOutput")

    with tile.TileContext(nc) as tc, ExitStack() as ctx:
        consts = ctx.enter_context(tc.tile_pool(name="consts", bufs=1))
        allp = ctx.enter_context(tc.tile_pool(name="allp", bufs=8))
        tpool = ctx.enter_context(tc.tile_pool(name="tpool", bufs=14))
        misc = ctx.enter_context(tc.tile_pool(name="misc", bufs=2))
        pa = ctx.enter_context(tc.tile_pool(name="pa", bufs=5, space="PSUM"))
        ps_fix = ctx.enter_context(tc.tile_pool(name="ps_fix", bufs=1, space="PSUM"))

        # ut + lt first (needed by round 0); big tail consts on other queues
        cba = consts.tile([128, 512], BF16, tag="c_cba")
        nc.scalar.dma_start(cba[:], CBA_d.ap())
        lt = consts.tile([128, 2 * b_loc], F32, tag="c_lt")
        nc.scalar.dma_start(lt[:], LT_d.ap())
        cbb = consts.tile([128, 640], BF16, tag="c_cbb")
        nc.gpsimd.dma_start(cbb[:], CBB_d.ap())
        cbf8 = consts.tile([128, 128, 2, 64], F8, tag="c_cbf8")
        nc.gpsimd.dma_start(cbf8[:], CBF8_d.ap())
        ut = cba[:, 0:512]
        mwp = cbb[:, 0:512]
        sel = cbb[0:GROUP, 512:640]

        def UT_ap(h, k):
            return ut[:, (2 * h + k) * 128:(2 * h + k + 1) * 128]

        def MW_ap(h, i):
            return mwp[:, (h * 8 + i) * 32:(h * 8 + i) * 32 + 32]

        def VS8_ap(hl, b):
            return cbf8[:, hl * 64 + b, :, :]

        # persistent psum: scores [b_loc, S], PT [128, 2S] (g in col halves)
        scores_ps = ps_fix.tile([b_loc, S], F32, tag="scps")
        PT2 = ps_fix.tile([128, 2 * S], F32, tag="pt")
        PT = [PT2[:, 0:S], PT2[:, S:2 * S]]

        # outputs (SBUF)
        numer = misc.tile([128, 2], F32, tag="numer", bufs=1)
        esum_sb = misc.tile([b_loc, 1], F32, tag="esum", bufs=1)

        pt_blocks = set()

        def emit_tail(r, at, tts):
            # score + P matmuls for the 4 batches of round r
            for u in range(2):
                for bb in range(2):
                    b = 4 * r + 2 * u + bb
                    bl = b % GROUP
                    g = b // GROUP
                    for hl in range(2):
                        nc.tensor.matmul(
                            scores_ps[:, :],
                            VS8_ap(hl, b),
                            tts[(u, bb)][:, :, :],
                            start=(b == 0 and hl == 0),
                            stop=(b == b_loc - 1 and hl == 1),
                            perf_mode=DR,
                            skip_group_check=True)
                    j32 = 32 * (bl // 8)
                    i8 = bl % 8
                    blk_new = (g, j32) not in pt_blocks
                    pt_blocks.add((g, j32))
                    for h in range(2):
                        nc.tensor.matmul(
                            PT[g][j32:j32 + 32, :], MW_ap(h, i8),
                            at[h][:, (2 * u + bb) * S:(2 * u + bb + 1) * S],
                            tile_position=(0, j32),
                            start=(blk_new and h == 0),
                            stop=(bl % 8 == 7 and h == 1),
                            skip_group_check=True)

        def endgame(g):
            g0 = g * GROUP
            e_sb = misc.tile([GROUP, S], BF16, tag="e")
            erep = ps_fix.tile([128, S], F32, tag="erep", name="erep")
            nc.scalar.activation(e_sb[:], scores_ps[g0:g0 + GROUP, :], AF.Exp)
            nc.vector.tensor_reduce(esum_sb[g0:g0 + GROUP, :], e_sb[:],
                                    axis=mybir.AxisListType.X, op=ALU.add)
            nc.tensor.matmul(erep[:], sel[:, 0:128], e_sb[:],
                             start=True, stop=True, skip_group_check=True)
            pt_sb = misc.tile([128, S], F32, tag="ptsb")
            nc.vector.tensor_copy(pt_sb[:], PT[g][:])
            scratch = misc.tile([128, S], F32, tag="scr")
            nc.vector.tensor_mul(scratch[:], pt_sb[:], erep[:])
            nc.vector.tensor_reduce(numer[:, g:g + 1], scratch[:],
                                    axis=mybir.AxisListType.X, op=ALU.add)

        pending = []
        for r in range(ROUNDS):
            at = []
            for h in range(2):
                a = allp.tile([128, 4 * S], BF16, tag="allp")
                nc.sync.dma_start(a[:], allT[h].ap()[:, r * 4 * S:(r + 1) * 4 * S])
                at.append(a)
            pas = {}
            for u in range(2):
                for k in range(2):
                    paT = pa.tile([128, 2 * S], F32, tag="pa")
                    for h in range(2):
                        nc.tensor.matmul(
                            paT[:], UT_ap(h, k), at[h][:, u * 2 * S:(u + 1) * 2 * S],
                            start=(h == 0), stop=(h == 1))
                    pas[(u, k)] = paT
            tts = {}
            for u in range(2):
                for bb in range(2):
                    tt = tpool.tile([128, 2, S], F8, tag="tpool")
                    b = 4 * r + 2 * u + bb
                    for k in range(2):
                        nc.scalar.activation(
                            tt[:, k, :],
                            pas[(u, k)][:, bb * S:(bb + 1) * S],
                            AF.Tanh, bias=lt[:, k * b_loc + b:k * b_loc + b + 1])
                    tts[(u, bb)] = tt
            pending.append((r, at, tts))
            while len(pending) > (2 if r < ROUNDS - 3 else 1):
                emit_tail(*pending.pop(0))
            if r == 10:
                endgame(0)
        for p in pending:
            emit_tail(*p)
        endgame(1)
        nc.sync.dma_start(numer_d.ap(), numer[:])
        nc.sync.dma_start(esum_d.ap(), esum_sb[:])
    nc.compile()
    return nc


def prep_core_inputs(all_c, last_c, U, W, V, MetaW, b_loc=B_LOC):
    x = np.ascontiguousarray(all_c.transpose(2, 0, 1)).astype(NBF)  # [H, b, S]
    m = {}
    m["allT0"] = np.ascontiguousarray(x[:128].reshape(128, b_loc * S))
    m["allT1"] = np.ascontiguousarray(x[128:].reshape(128, b_loc * S))
    l = (last_c @ W.T).astype(np.float32)
    m["LT"] = np.ascontiguousarray(
        l.T.reshape(2, 128, b_loc).transpose(1, 0, 2).reshape(128, 2 * b_loc))
    ut = U.reshape(2, 128, 2, 128).transpose(3, 2, 0, 1).reshape(128, 512)
    mwp = np.zeros((128, 2, 8, 32), np.float32)        # baseline packed MetaW
    for h in range(2):
        for i in range(8):
            mwp[:, h, i, 4 * i:4 * i + 4] = MetaW[:, 128 * h:128 * (h + 1)].T
    mwp = mwp.reshape(128, 512)
    sel = np.zeros((128, 128), np.float32)             # [b, p] = 1 if b == p//4
    for p in range(128):
        sel[p // 4, p] = 1.0
    m["CBA"] = np.ascontiguousarray(ut).astype(NBF)
    m["CBB"] = np.ascontiguousarray(
        np.concatenate([mwp, sel], axis=1)).astype(NBF)
    # fp8 hi-lo V selector stationaries: slot (hl*32+j) -> [128, 2(k), 32]
    v = V[:, 0].reshape(2, 128).T.astype(np.float32)   # [128, 2]
    vhi = v.astype(NF8).astype(np.float32)
    vlo = (v - vhi).astype(NF8).astype(np.float32)
    vsel8 = np.zeros((128, 128, 2, 64), np.float32)
    for hl, vv in enumerate([vhi, vlo]):
        for b in range(64):
            for k in range(2):
                vsel8[:, hl * 64 + b, k, b] = vv[:, k]
    m["CBF8"] = np.ascontiguousarray(vsel8).astype(NF8)
    return m


def postprocess_core(numer, esum, Metab, b_loc=B_LOC):
    out = np.empty((b_loc, 4), np.float32)
    for g in range(2):
        out[g * GROUP:(g + 1) * GROUP] = numer[:4 * GROUP, g].reshape(GROUP, 4)
    return out / esum.reshape(b_loc, 1) + Metab.reshape(1, 4)


_cache = {}


def _get_nc():
    if "nc" not in _cache:
        _cache["nc"] = build_nc(B_LOC)
    return _cache["nc"]


def kernel(all_memory, last_memory, U, W, V, MetaW, Metab):
    all_memory = np.asarray(all_memory, dtype=np.float32)
    last_memory = np.asarray(last_memory, dtype=np.float32)
    U = np.asarray(U, dtype=np.float32)
    W = np.asarray(W, dtype=np.float32)
    V = np.asarray(V, dtype=np.float32)
    MetaW = np.asarray(MetaW, dtype=np.float32)
    Metab = np.asarray(Metab, dtype=np.float32)
    nc = _get_nc()
    in_maps = []
    for c in range(N_CORES):
        sl = slice(c * B_LOC, (c + 1) * B_LOC)
        in_maps.append(prep_core_inputs(
            all_memory[sl], last_memory[sl], U, W, V, MetaW))
    res = run_bass_kernel_spmd(nc, in_maps, core_ids=list(range(N_CORES)))
    outs = [postprocess_core(res.results[c]["numer"], res.results[c]["esum"],
                             Metab) for c in range(N_CORES)]
    return np.concatenate(outs, axis=0).astype(np.float32)


# revision 35
# speedup vs baseline: 1.2998x; 1.2998x over previous
"""Additive-attention pooling (nn_Meta_Module) Trainium2 kernel.

Full inputs in, full output out. Internally: pure data-parallel over 8
NeuronCores (batch 512 -> 64/core). Per core, a Bass/Tile kernel computes
  a   = all_memory @ U.T            (PE, bf16, [k,(b,s)] layout)
  t   = tanh(a + last @ W.T)        (ScalarE, per-batch per-partition bias)
  sc  = V.T @ t                     (PE, V-in-col-j selector stationary,
                                     scores land [batch, s] in PSUM)
  P   = all_memory @ MetaW.T        (PE, packed [128,32] MW stationary)
  e   = exp(sc), esum               (ScalarE + DVE reduce)
  numer = sum_s e*P                 (PE replicate + DVE mul/reduce)
Outputs are DVE-block-transposed so the result DMA is a few fat
descriptors instead of one per partition. Host: numer/esum + Metab.
"""
import numpy as np
import ml_dtypes
from contextlib import ExitStack

import concourse.bass as bass
import concourse.tile as tile
import concourse.mybir as mybir
from concourse import bacc
from concourse.bass_utils import run_bass_kernel_spmd

BF16 = mybir.dt.bfloat16
F32 = mybir.dt.float32
AF = mybir.ActivationFunctionType
ALU = mybir.AluOpType
NBF = ml_dtypes.bfloat16

B, S, H = 512, 200, 256
N_CORES = 8
B_LOC = B // N_CORES          # 64 batches per core
GROUP = 32                    # batches per scores/PT block


def build_nc(b_loc=B_LOC, debug=False):
    ROUNDS = b_loc // 4
    nc = bacc.Bacc("TRN2", target_bir_lowering=False, debug=debug)

    def din(name, shape, dt=BF16):
        return nc.dram_tensor(name, shape, dt, kind="ExternalInput")

    allT = [din(f"allT{h}", [128, b_loc * S]) for h in range(2)]
    CBA_d = din("CBA", [128, 512])
    CBB_d = din("CBB", [128, 512 + 2048 + 128])
    LT_d = din("LT", [128, 2 * b_loc], F32)
    numerT_d = nc.dram_tensor("numerT", [8, 32], F32, kind="ExternalOutput")
    esumT_d = nc.dram_tensor("esumT", [2, 32], F32, kind="ExternalOutput")

    with tile.TileContext(nc) as tc, ExitStack() as ctx:
        consts = ctx.enter_context(tc.tile_pool(name="consts", bufs=1))
        allp = ctx.enter_context(tc.tile_pool(name="allp", bufs=8))
        tpool = ctx.enter_context(tc.tile_pool(name="tpool", bufs=14))
        misc = ctx.enter_context(tc.tile_pool(name="misc", bufs=2))
        pa = ctx.enter_context(tc.tile_pool(name="pa", bufs=5, space="PSUM"))
        ps_fix = ctx.enter_context(tc.tile_pool(name="ps_fix", bufs=1, space="PSUM"))

        # ut + lt first (needed by round 0); big tail consts on gpsimd queue
        cba = consts.tile([128, 512], BF16, tag="c_cba")
        nc.scalar.dma_start(cba[:], CBA_d.ap())
        lt = consts.tile([128, 2 * b_loc], F32, tag="c_lt")
        nc.scalar.dma_start(lt[:], LT_d.ap())
        cbb = consts.tile([128, 2688], BF16, tag="c_cbb")
        nc.gpsimd.dma_start(cbb[:], CBB_d.ap())
        ut = cba[:, 0:512]
        mwp = cbb[:, 0:512]
        vsel = cbb[:, 512:2560]
        sel = cbb[0:GROUP, 2560:2688]

        def UT_ap(h, k):
            return ut[:, (2 * h + k) * 128:(2 * h + k + 1) * 128]

        def MW_ap(h, i):
            return mwp[:, (h * 8 + i) * 32:(h * 8 + i) * 32 + 32]

        def VS_ap(k, j):
            return vsel[:, (k * 32 + j) * 32:(k * 32 + j) * 32 + 32]

        # persistent psum: scores [b_loc, S], PT [128, 2S] (g in col halves)
        scores_ps = ps_fix.tile([b_loc, S], F32, tag="scps")
        PT2 = ps_fix.tile([128, 2 * S], F32, tag="pt")
        PT = [PT2[:, 0:S], PT2[:, S:2 * S]]

        # outputs (SBUF): padded for DVE 32x32 block transpose
        numer = misc.tile([128, 32], F32, tag="numer", bufs=1)
        esum_sb = misc.tile([b_loc, 32], F32, tag="esum", bufs=1)
        numerT = misc.tile([128, 32], F32, tag="numerT", bufs=1)
        esumT = misc.tile([b_loc, 32], F32, tag="esumT", bufs=1)
        nc.gpsimd.memset(numer[:], 0.0)
        nc.gpsimd.memset(esum_sb[:], 0.0)

        pt_blocks = set()

        def emit_tail(r, at, tts):
            # score + P matmuls for the 4 batches of round r
            for u in range(2):
                for bb in range(2):
                    b = 4 * r + 2 * u + bb
                    bl = b % GROUP
                    g = b // GROUP
                    for k in range(2):
                        nc.tensor.matmul(
                            scores_ps[g * GROUP:(g + 1) * GROUP, :],
                            VS_ap(k, bl),
                            tts[(u, k)][:, bb * S:(bb + 1) * S],
                            start=(bl == 0 and k == 0),
                            stop=(bl == GROUP - 1 and k == 1),
                            skip_group_check=True)
                    j32 = 32 * (bl // 8)
                    i8 = bl % 8
                    blk_new = (g, j32) not in pt_blocks
                    pt_blocks.add((g, j32))
                    for h in range(2):
                        nc.tensor.matmul(
                            PT[g][j32:j32 + 32, :], MW_ap(h, i8),
                            at[h][:, (2 * u + bb) * S:(2 * u + bb + 1) * S],
                            tile_position=(0, j32),
                            start=(blk_new and h == 0),
                            stop=(bl % 8 == 7 and h == 1),
                            skip_group_check=True)

        def endgame(g):
            g0 = g * GROUP
            e_sb = misc.tile([GROUP, S], BF16, tag="e")
            erep = ps_fix.tile([128, S], F32, tag="erep", name="erep")
            nc.scalar.activation(e_sb[:], scores_ps[g0:g0 + GROUP, :], AF.Exp)
            nc.vector.tensor_reduce(esum_sb[g0:g0 + GROUP, 0:1], e_sb[:],
                                    axis=mybir.AxisListType.X, op=ALU.add)
            nc.tensor.matmul(erep[:], sel[:, 0:128], e_sb[:],
                             start=True, stop=True, skip_group_check=True)
            pt_sb = misc.tile([128, S], F32, tag="ptsb")
            nc.vector.tensor_copy(pt_sb[:], PT[g][:])
            scratch = misc.tile([128, S], F32, tag="scr")
            nc.vector.tensor_mul(scratch[:], pt_sb[:], erep[:])
            nc.vector.tensor_reduce(numer[:, g:g + 1], scratch[:],
                                    axis=mybir.AxisListType.X, op=ALU.add)

        pending = []
        for r in range(ROUNDS):
            at = []
            for h in range(2):
                a = allp.tile([128, 4 * S], BF16, tag="allp")
                nc.sync.dma_start(a[:], allT[h].ap()[:, r * 4 * S:(r + 1) * 4 * S])
                at.append(a)
            pas = {}
            for u in range(2):
                for k in range(2):
                    paT = pa.tile([128, 2 * S], F32, tag="pa")
                    for h in range(2):
                        nc.tensor.matmul(
                            paT[:], UT_ap(h, k), at[h][:, u * 2 * S:(u + 1) * 2 * S],
                            start=(h == 0), stop=(h == 1))
                    pas[(u, k)] = paT
            tts = {}
            for u in range(2):
                for k in range(2):
                    tt = tpool.tile([128, 2 * S], BF16, tag="tpool")
                    for bb in range(2):
                        b = 4 * r + 2 * u + bb
                        nc.scalar.activation(
                            tt[:, bb * S:(bb + 1) * S],
                            pas[(u, k)][:, bb * S:(bb + 1) * S],
                            AF.Tanh, bias=lt[:, k * b_loc + b:k * b_loc + b + 1])
                    tts[(u, k)] = tt
            pending.append((r, at, tts))
            while len(pending) > (2 if r < ROUNDS - 3 else 1):
                emit_tail(*pending.pop(0))
            if r == 10:
                endgame(0)
        for p in pending:
            emit_tail(*p)
        endgame(1)
        # block-transpose outputs so the result DMA is a few fat descriptors
        nc.vector.transpose(numerT[:], numer[:])
        nc.vector.transpose(esumT[:], esum_sb[:])
        for i in range(4):
            nc.sync.dma_start(numerT_d.ap()[2 * i:2 * i + 2, :],
                              numerT[32 * i:32 * i + 2, 0:32])
        for i in range(2):
            nc.sync.dma_start(esumT_d.ap()[i:i + 1, :],
                              esumT[32 * i:32 * i + 1, 0:32])
    nc.compile()
    return nc


def prep_core_inputs(all_c, last_c, U, W, V, MetaW, b_loc=B_LOC):
    x = np.ascontiguousarray(all_c.transpose(2, 0, 1)).astype(NBF)  # [H, b, S]
    m = {}
    m["allT0"] = np.ascontiguousarray(x[:128].reshape(128, b_loc * S))
    m["allT1"] = np.ascontiguousarray(x[128:].reshape(128, b_loc * S))
    l = (last_c @ W.T).astype(np.float32)
    m["LT"] = np.ascontiguousarray(
        l.T.reshape(2, 128, b_loc).transpose(1, 0, 2).reshape(128, 2 * b_loc))
    ut = U.reshape(2, 128, 2, 128).transpose(3, 2, 0, 1).reshape(128, 512)
    mwp = np.zeros((128, 2, 8, 32), np.float32)        # baseline packed MetaW
    for h in range(2):
        for i in range(8):
            mwp[:, h, i, 4 * i:4 * i + 4] = MetaW[:, 128 * h:128 * (h + 1)].T
    mwp = mwp.reshape(128, 512)
    v = V[:, 0].reshape(2, 128).T                      # [128, 2]
    vsel = np.zeros((128, 2, 32, 32), np.float32)      # V in col j, zeros else
    for k in range(2):
        for j in range(32):
            vsel[:, k, j, j] = v[:, k]
    vsel = vsel.reshape(128, 2048)
    sel = np.zeros((128, 128), np.float32)             # [b, p] = 1 if b == p//4
    for p in range(128):
        sel[p // 4, p] = 1.0
    m["CBA"] = np.ascontiguousarray(ut).astype(NBF)
    m["CBB"] = np.ascontiguousarray(
        np.concatenate([mwp, vsel, sel], axis=1)).astype(NBF)
    return m


def postprocess_core(numerT, esumT, Metab, b_loc=B_LOC):
    # numerT[2*i + r, c] = numer[32*i + c, r]; numer row 4*bl+jj, col g
    numer = np.empty((128, 2), np.float32)
    for i in range(4):
        numer[32 * i:32 * i + 32, :] = numerT[2 * i:2 * i + 2, :].T
    esum = np.empty((b_loc,), np.float32)
    for i in range(2):
        esum[32 * i:32 * i + 32] = esumT[i]
    out = np.empty((b_loc, 4), np.float32)
    for g in range(2):
        out[g * GROUP:(g + 1) * GROUP] = numer[:4 * GROUP, g].reshape(GROUP, 4)
    return out / esum.reshape(b_loc, 1) + Metab.reshape(1, 4)


_cache = {}


def _get_nc():
    if "nc" not in _cache:
        _cache["nc"] = build_nc(B_LOC)
    return _cache["nc"]


def kernel(all_memory, last_memory, U, W, V, MetaW, Metab):
    all_memory = np.asarray(all_memory, dtype=np.float32)
    last_memory = np.asarray(last_memory, dtype=np.float32)
    U = np.asarray(U, dtype=np.float32)
    W = np.asarray(W, dtype=np.float32)
    V = np.asarray(V, dtype=np.float32)
    MetaW = np.asarray(MetaW, dtype=np.float32)
    Metab = np.asarray(Metab, dtype=np.float32)
    nc = _get_nc()
    in_maps = []
    for c in range(N_CORES):
        sl = slice(c * B_LOC, (c + 1) * B_LOC)
        in_maps.append(prep_core_inputs(
            all_memory[sl], last_memory[sl], U, W, V, MetaW))
    res = run_bass_kernel_spmd(nc, in_maps, core_ids=list(range(N_CORES)))
    outs = [postprocess_core(res.results[c]["numerT"], res.results[c]["esumT"],
                             Metab) for c in range(N_CORES)]
    return np.concatenate(outs, axis=0).astype(np.float32)
